# revision 66
# baseline (speedup 1.0000x reference)
"""Trainium2 Bass kernel for sparse (causal, tanh-clamped) attention.

Problem: B=2, L=2048, D=1024, H=16 heads x 64 dim; S = QK^T/8;
S = 30*tanh(S); causal + attention_mask; softmax; out = attn @ V.

Sharding: 2 heads per core across 8 cores (tensor-parallel on heads).

The ACT (scalar) engine is the bottleneck: tanh+exp over every causal
score column is ~116us of irreducible element time; everything else is
arranged to keep ACT saturated and to minimize its per-instruction
overhead (~330ns per tanh/exp pair).

Design:
 - fp16 x/W/Q/K (precision for the score path: the exp amplifies score
   errors by d(30*tanh)/ds), bf16 probabilities/V (P can be ~e^-60;
   fp16 would flush it to 0 and NaN the softmax denominator). All
   matmuls cost 1 cycle/moving-row at 16-bit; DMA bytes are halved.
 - S^T[k, q] layout throughout; no P transpose (P^T feeds AV directly).
 - attention_mask folded into the score matmul via a 65th contraction
   row; causal diagonal masked by a tril MULTIPLY on the probabilities
   (an additive -big mask would leave e^-60 ghosts that dominate rows
   whose live probabilities are comparably small).
 - bounded logits: P = exp(30*tanh(s)-30), no running max; denominator
   comes free as a ones-column in the augmented V tile (po row 0).
 - V projected token-major directly (stationary=x chunk, moving=W_V
   chunk): no V transposes, no separate vaug copies.
 - merged-head spans (qw<=512): one [128, <=1024] PSUM strip holds both
   heads' score columns per k-tile, so a single tanh/exp pair covers
   both heads -- 82 ACT pairs total instead of 102.
 - AV accumulation per 128-column tile with stop at ki==j; bank drains
   to SBUF when retired, transpose/normalize chain deferred into the
   next span (psO-ring aliasing requires pts after po's last access).
   The last span drains per-128 with pt from the strip ring and
   per-tile stores on the sync queue for a ~4us tail.
 - software-pipelined emission: score(ki+1) prefetched ahead of AV/pump
   on the in-order PE queue; background work (projection chunks, V
   tiles, x loads, stores) flows through a budget-limited pump with
   explicit prerequisites, forced just-in-time at span boundaries with
   drains split across ACT/DVE when ACT would be idle anyway.
"""

import sys

if "/opt/trn_rl_repo" not in sys.path:
    sys.path.insert(0, "/opt/trn_rl_repo")

import numpy as np

B = 2
L = 2048
D = 1024
H = 16
DH = 64
N_CORES = 8
T = B * L            # 4096 tokens
E = 128              # per-core output features (2 heads)
ND = D // 128        # 8 contraction chunks
NEG_BIG = 6.0e4   # fp16-safe; still saturates tanh
TAU = 30.0

_CACHE = {}


def _build_program():
    import concourse.bacc as bacc
    import concourse.tile as tile
    from concourse import mybir
    from collections import deque

    F32 = mybir.dt.float32
    F16 = mybir.dt.float16
    BF16 = mybir.dt.bfloat16
    AF = mybir.ActivationFunctionType

    nc = bacc.Bacc("TRN2", target_bir_lowering=False, debug=False,
                   num_devices=N_CORES)

    # --- DRAM tensors -----------------------------------------------------
    # x host-packed: group0 of each batch split in two 256-token pieces
    # laid out [p, (d, t)], remaining groups [p, (g, d, t512)].
    xh0a_d = [nc.dram_tensor(f"xh0a{b}", [128, ND * 256], F16,
                             kind="ExternalInput") for b in range(B)]
    xh0b_d = [nc.dram_tensor(f"xh0b{b}", [128, ND * 256], F16,
                             kind="ExternalInput") for b in range(B)]
    xhr_d = nc.dram_tensor("xhr", [128, 6 * ND * 512], F16,
                           kind="ExternalInput")
    wqk_d = nc.dram_tensor("wqk", [128, 2 * ND * 128], F16,
                           kind="ExternalInput")
    wv_d = nc.dram_tensor("wv", [128, ND * 128], F16,
                          kind="ExternalInput")
    kaug_d = nc.dram_tensor("kaug", [1, T], F16, kind="ExternalInput")
    trilb_d = nc.dram_tensor("trilb", [128, 128], BF16,
                             kind="ExternalInput")
    identf_d = nc.dram_tensor("identf", [128, 128], F32,
                              kind="ExternalInput")
    out_d = nc.dram_tensor("out", [B, L, E], F32, kind="ExternalOutput")

    NJ = L // 128     # 16 column tiles per sequence

    with tile.TileContext(nc) as tc:
        with (
            tc.tile_pool(name="const", bufs=1) as constp,
            tc.tile_pool(name="weights", bufs=1) as wp,
            tc.tile_pool(name="qk", bufs=1) as qkp,
            tc.tile_pool(name="va", bufs=1) as vap,
            tc.tile_pool(name="x0", bufs=4) as x0p,
            tc.tile_pool(name="xg", bufs=6) as xgp,
            tc.tile_pool(name="prob", bufs=6) as ppp,
            tc.tile_pool(name="epi", bufs=6) as epip,
            tc.tile_pool(name="ostage", bufs=1) as ostagep,
            tc.tile_pool(name="strip", bufs=3, space="PSUM") as stripp,
            tc.tile_pool(name="psO", bufs=1, space="PSUM") as pop,
        ):
            # --- constants ----------------------------------------------
            trilb_t = constp.tile([128, 128], BF16, tag="trilb")
            identf_t = constp.tile([128, 128], F32, tag="identf")
            n30_t = constp.tile([128, 1], F32, tag="n30")
            wz_t = constp.tile([128, 128], F16, tag="wzero")
            nc.gpsimd.memset(wz_t[:], 0.0)

            wqk_t = wp.tile([128, 2 * ND * 128], F16, tag="wqk",
                            name="wqk")
            wv_t = wp.tile([128, ND * 128], F16, tag="wv", name="wv")

            QT = [[qkp.tile([65, L], F16, tag=f"qt{h}{b}",
                            name=f"qt{h}{b}")
                   for b in range(B)] for h in range(2)]
            KT = [[qkp.tile([65, L], F16, tag=f"kt{h}{b}",
                            name=f"kt{h}{b}")
                   for b in range(B)] for h in range(2)]
            # va[b][ki]: [128 tokens, 131]: col0=ones, 1:65=h0 feats,
            # col65=ones, 66:130=h1 feats
            VA = [[vap.tile([128, 131], BF16, tag=f"va{b}_{k}",
                            name=f"va{b}_{k}") for k in range(NJ)]
                  for b in range(B)]
            OST = [[ostagep.tile([128, 512], F32, tag=f"os{b}_{k}",
                                 name=f"os{b}_{k}") for k in range(4)]
                   for b in range(B)]

            # aug rows for batch 0 first: the first scores read QT
            # row 64, and these [1, 2048] memsets cost ~1.8us each on
            # the serial Pool queue
            for h in range(2):
                nc.gpsimd.memset(QT[h][0][64:65, :], 1.0)
            nc.gpsimd.memset(n30_t[:], -TAU)
            # ones columns of VA (written once; Pool is idle)
            for b in range(B):
                for k in range(NJ):
                    nc.gpsimd.memset(VA[b][k][:, 0:1], 1.0)
                    nc.gpsimd.memset(VA[b][k][:, 65:66], 1.0)
            for h in range(2):
                nc.gpsimd.memset(QT[h][1][64:65, :], 1.0)

            # --- initial DMAs. The startup chain is HWDGE-bound
            # (~625ns per dma_start on a serial device), so the minimum
            # number of transfers gates the first projection.
            x0_tiles = {}   # (b, half) -> tile
            x0_tiles[(0, 0)] = x0p.tile([128, ND * 256], F16, tag="x0",
                                        name="x00")
            nc.sync.dma_start(x0_tiles[(0, 0)][:], xh0a_d[0].ap()[:])
            nc.sync.dma_start(wqk_t[:, 0:1024], wqk_d.ap()[:, 0:1024])
            nc.sync.dma_start(wqk_t[:, 1024:2048],
                              wqk_d.ap()[:, 1024:2048])
            for h in range(2):
                nc.sync.dma_start(KT[h][0][64:65, :], kaug_d.ap()[0:1, 0:L])
            x0_tiles[(0, 1)] = x0p.tile([128, ND * 256], F16, tag="x0",
                                        name="x01")
            nc.sync.dma_start(x0_tiles[(0, 1)][:], xh0b_d[0].ap()[:])
            nc.sync.dma_start(trilb_t[:], trilb_d.ap()[:])
            nc.sync.dma_start(wv_t[:], wv_d.ap()[:])
            nc.sync.dma_start(identf_t[:], identf_d.ap()[:])

            # PE warmup: ramp the p-state and keep PE busy until the
            # first projection inputs land (a gap resets the ramp).
            wm = stripp.tile([128, 1024], F32, tag="strip",
                             name="warm")
            for _ in range(20):
                nc.tensor.matmul(wm[:, 0:128], wz_t[:], wz_t[:],
                                 start=True, stop=True)

            xg_tiles = {}   # group g (1..7) -> tile

            def load_group(g):
                t = xgp.tile([128, ND * 512], F16, tag="xg",
                             name=f"xg{g}")
                idx = g - 1 if g <= 3 else g - 2   # xhr: groups 1,2,3,5,6,7
                nc.sync.dma_start(
                    t[:], xhr_d.ap()[:, idx * 4096:(idx + 1) * 4096])
                xg_tiles[g] = t

            def load_aug_b1():
                for h in range(2):
                    nc.sync.dma_start(KT[h][1][64:65, :],
                                      kaug_d.ap()[0:1, L:2 * L])

            def x_slice(b, t0, width):
                """SBUF source tile for tokens [t0, t0+width) of batch b:
                returns (tile, per-d stride, column base)."""
                tglob = b * L + t0
                g = tglob // 512
                if g in (0, 4):   # each batch's first group: split tiles
                    half = (tglob % 512) // 256
                    return x0_tiles[(b, half)], 256, tglob % 256
                return xg_tiles[g], 512, tglob % 512

            def load_b1_first():
                for half in range(2):
                    t = x0p.tile([128, ND * 256], F16, tag="x0",
                                 name=f"x1{half}")
                    nc.sync.dma_start(t[:], (xh0a_d[1] if half == 0
                                             else xh0b_d[1]).ap()[:])
                    x0_tiles[(1, half)] = t

            # --- projection emitters ------------------------------------
            def qk_chunk_mm(b, c, dlo, dhi, pj):
                """QK projection matmuls for 256-token chunk c of batch
                b, d-chunks [dlo, dhi)."""
                t0 = 256 * c
                xt, tw, base = x_slice(b, t0, 256)
                for d in range(dlo, dhi):
                    xs = xt[:, d * tw + base:d * tw + base + 256]
                    nc.tensor.matmul(
                        pj[:, 0:256], wqk_t[:, d * 128:(d + 1) * 128],
                        xs, start=(d == 0), stop=(d == ND - 1))
                    nc.tensor.matmul(
                        pj[:, 256:512],
                        wqk_t[:, ND * 128 + d * 128:ND * 128 + (d + 1) * 128],
                        xs, start=False, stop=(d == ND - 1))

            def qk_chunk_drain(b, c, part, pj, on_act=False, hs=(0, 1)):
                """Drain chunk c: part 0 = Q, part 1 = K; heads hs."""
                t0 = 256 * c
                dst = QT if part == 0 else KT
                for h in hs:
                    src = pj[h * 64:(h + 1) * 64,
                             part * 256:part * 256 + 256]
                    d = dst[h][b][0:64, t0:t0 + 256]
                    if on_act:
                        nc.scalar.activation(d, src, AF.Identity)
                    else:
                        nc.vector.tensor_copy(d, src)

            def v_tile(b, ki):
                """V projection for token-tile ki of batch b:
                token-major via stationary-x matmul; drains into VA."""
                t0 = 128 * ki
                xt, tw, base = x_slice(b, t0, 128)
                pv = stripp.tile([128, 1024], F32, tag="strip",
                                 name="pv")
                for d in range(ND):
                    nc.tensor.matmul(
                        pv[:, 0:128],
                        xt[:, d * tw + base:d * tw + base + 128],
                        wv_t[:, d * 128:(d + 1) * 128],
                        start=(d == 0), stop=(d == ND - 1))
                nc.vector.tensor_copy(VA[b][ki][:, 1:65], pv[:, 0:64])
                nc.vector.tensor_copy(VA[b][ki][:, 66:130], pv[:, 64:128])

            # --- background queue ---------------------------------------
            bg = deque()
            emitted = set()
            by_key = {}

            def bg_add(key, pe, dve, fn, requires=()):
                it = {"key": key, "pe": pe, "dve": dve, "fn": fn,
                      "req": tuple(requires)}
                bg.append(it)
                by_key[key] = it

            def bg_run(item):
                if item["key"] in emitted:
                    return
                for r in item["req"]:
                    if r not in emitted and r in by_key:
                        bg_run(by_key[r])
                emitted.add(item["key"])
                item["fn"]()

            def pump(pe_budget, dve_budget):
                # scan past blocked items (requirements make out-of-order
                # emission safe); bounded lookahead keeps need-order bias.
                # At most one PSUM-using (PE) quantum per call: a second
                # would cycle the strip ring into a head-of-line stall.
                scanned = 0
                pe_quanta = 0
                i = 0
                while i < len(bg) and scanned < 12:
                    it = bg[i]
                    if it["key"] in emitted:
                        del bg[i]
                        continue
                    scanned += 1
                    fits = (it["pe"] <= pe_budget
                            and it["dve"] <= dve_budget
                            and (it["pe"] == 0 or pe_quanta < 1))
                    if fits:
                        del bg[i]
                        bg_run(it)
                        pe_budget -= it["pe"]
                        dve_budget -= it["dve"]
                        if it["pe"] > 0:
                            pe_quanta += 1
                    else:
                        i += 1

            def force(pred):
                # emit exactly the matching items (plus their declared
                # prerequisites), leaving unrelated queued work in place
                for it in list(bg):
                    if it["key"] not in emitted and pred(it["key"]):
                        bg_run(it)
                while bg and bg[0]["key"] in emitted:
                    bg.popleft()

            def x_req(b, t0):
                g = (b * L + t0) // 512
                if g == 0:
                    return ()
                if g == 4:
                    return (("x1a", 0),)
                return (("xg", g),)

            drain_fns = {}

            def queue_qk(b, c, act_drain=False):
                shared = {}
                req = x_req(b, 256 * c)

                def mk_mm(dlo, dhi):
                    def fn():
                        if "pj" not in shared:
                            shared["pj"] = stripp.tile([128, 1024], F32,
                                                       tag="strip",
                                                       name="pj")
                        qk_chunk_mm(b, c, dlo, dhi, shared["pj"])
                    return fn

                def mk_drain(part, h):
                    def fn(on_act=False):
                        qk_chunk_drain(b, c, part, shared["pj"],
                                       on_act=on_act, hs=(h,))
                    return fn
                for q in range(4):
                    bg_add(("qkm", b, c, q), 440, 0,
                           mk_mm(q * 2, q * 2 + 2),
                           req if q == 0 else (("qkm", b, c, q - 1),))
                for part in range(2):
                    for h in range(2):
                        fn = mk_drain(part, h)
                        drain_fns[(b, c, part, h)] = fn
                        bg_add(("qkd", b, c, part, h), 0, 400, fn,
                               (("qkm", b, c, 3),))

            def queue_v(b, ki):
                bg_add(("v", b, ki), 430, 390,
                       lambda b=b, ki=ki: v_tile(b, ki),
                       x_req(b, 128 * ki))

            def queue_dma(key, fn, requires=()):
                bg_add(key, 0, 0, fn, requires)

            # --- attention ----------------------------------------------
            def queue_store(b, blk):
                def fn(b=b, blk=blk):
                    nc.gpsimd.dma_start(
                        out_d.ap()[b, blk * 512:(blk + 1) * 512, :]
                        .rearrange("(j p) e -> p j e", p=128),
                        OST[b][blk][:].rearrange("p (j e) -> p j e", j=4))
                queue_dma(("store", b, blk), fn)

            def normalize_tile(b, h, j, dst):
                """dst: transposed [128, 65] PSUM view (col0 = denom)."""
                rec = epip.tile([128, 1], F32, tag="rec", name="rec")
                nc.vector.reciprocal(rec[:], dst[0:128, 0:1])
                blk, j_in = j // 4, j % 4
                nc.vector.tensor_scalar_mul(
                    OST[b][blk][:, j_in * 128 + h * 64:
                                j_in * 128 + h * 64 + 64],
                    dst[0:128, 1:65], rec[:])
                if h == 1:
                    if b == 1 and blk == 3:
                        # final block: per-tile stores on the sync queue
                        # (HWDGE; the SWDGE gen on Pool would serialize
                        # the last three stores at ~1us each)
                        def fn(b=b, j=j, j_in=j_in, blk=blk):
                            nc.sync.dma_start(
                                out_d.ap()[b, j * 128:(j + 1) * 128, :],
                                OST[b][blk][:, j_in * 128:
                                            (j_in + 1) * 128])
                        queue_dma(("store", b, blk, j), fn)
                    elif j % 4 == 3:
                        queue_store(b, blk)

            def attention_span(b, qlo, qw, last_span=False,
                               carry=None):
                """Merged-head span: one [128, <=1024] strip holds both
                heads' score columns for each k-tile; a single tanh/exp
                pair covers them (halves the ACT instruction count).
                qw <= 512. Head 1's block sits at offset `h1b`:
                packed at w when 2w <= 512, else at 512 (bank B)."""
                jlo, jhi = qlo // 128, (qlo + qw) // 128
                state = {0: None, 1: None}
                prev = None
                bank_ots = {}   # h -> ot tile for the span's bank

                def get_po(h):
                    if state[h] is None:
                        state[h] = pop.tile([65, 512], F32,
                                            tag=f"po{h}", name=f"po{h}")
                    return state[h]

                sc = {}

                def emit_score(ki):
                    q0 = max(qlo, ki * 128)
                    w = qlo + qw - q0
                    h1b = w if 2 * w <= 512 else 512
                    strip = stripp.tile([128, 1024], F32, tag="strip",
                                        name="strip")
                    for h in range(2):
                        nc.tensor.matmul(
                            strip[:, h * h1b:h * h1b + w],
                            KT[h][b][:, ki * 128:ki * 128 + 128],
                            QT[h][b][:, q0:q0 + w],
                            start=True, stop=True)
                    sc[ki] = (strip, q0, w, h1b)

                def emit_av(ki, pp, q0, w, h1b):
                    for h in range(2):
                        po = get_po(h)
                        for j in range(max(ki, jlo), jhi):
                            cl = h * h1b + j * 128 - q0
                            nc.tensor.matmul(
                                po[:, j * 128 - qlo:(j + 1) * 128 - qlo],
                                VA[b][ki][:, h * 65:h * 65 + 65],
                                pp[:, cl:cl + 128],
                                start=(ki == 0 and j == jlo),
                                stop=(ki == j))
                            if ki != j:
                                continue
                            if last_span and j >= jhi - 4:
                                ot = epip.tile([65, 512], F32,
                                               tag="ot", name="ot")
                                nc.vector.tensor_copy(
                                    ot[0:65, 0:128],
                                    po[:, j * 128 - qlo:
                                        (j + 1) * 128 - qlo])
                                pt = stripp.tile([128, 1024], F32,
                                                 tag="strip", name="pt")
                                nc.tensor.transpose(
                                    pt[0:128, 0:65], ot[0:65, 0:128],
                                    identf_t[0:65, 0:65])
                                normalize_tile(b, h, j,
                                               pt[0:128, 0:65])
                            elif j == jhi - 1:
                                nb = jhi - jlo
                                ot = epip.tile([65, 512], F32,
                                               tag="ot", name="ot")
                                nc.vector.tensor_copy(
                                    ot[0:65, 0:nb * 128],
                                    po[:, 0:nb * 128])
                                bank_ots[h] = (ot, jlo, nb)

                emit_score(0)
                for ki in range(jhi):
                    if ki + 1 < jhi:
                        emit_score(ki + 1)
                    strip, q0, w, h1b = sc.pop(ki)
                    diag = (q0 == ki * 128)
                    tot = h1b + w
                    pp = ppp.tile([128, 1024], BF16, tag="pp",
                                  name="pp")
                    if w < h1b:
                        # gapped layout (h1 at offset 512): process both
                        # blocks with one 3D AP, skipping the hole
                        sv = strip[:].rearrange("p (g c) -> p g c",
                                                g=2)[:, :, 0:w]
                        pv = pp[:].rearrange("p (g c) -> p g c",
                                             g=2)[:, :, 0:w]
                        nc.scalar.activation(sv, sv, AF.Tanh,
                                             scale=0.125)
                        nc.scalar.activation(pv, sv, AF.Exp,
                                             bias=n30_t[:], scale=TAU)
                    else:
                        nc.scalar.activation(strip[:, 0:tot],
                                             strip[:, 0:tot],
                                             AF.Tanh, scale=0.125)
                        nc.scalar.activation(pp[:, 0:tot],
                                             strip[:, 0:tot],
                                             AF.Exp, bias=n30_t[:],
                                             scale=TAU)
                    if diag:
                        # exact-zero the below-diagonal ghosts
                        nc.vector.tensor_mul(pp[:, 0:128],
                                             pp[:, 0:128], trilb_t[:])
                        nc.vector.tensor_mul(
                            pp[:, h1b:h1b + 128],
                            pp[:, h1b:h1b + 128], trilb_t[:])
                    if carry is not None:
                        carry()
                        carry = None
                    if prev is not None:
                        if not ("v", b, prev[0]) in emitted:
                            force(lambda k, kk=prev[0]:
                                  k == ("v", b, kk))
                        emit_av(*prev)
                    act_ns = 1.67 * tot + 330
                    pe_ns = 0.43 * tot + (120 if diag else 0) + \
                        0.86 * 128 * max(0, jhi - max(ki - 1, jlo)) + 100
                    pump(max(0.0, act_ns - pe_ns - 100),
                         max(0.0, act_ns - 550))
                    prev = (ki, pp, q0, w, h1b)
                if prev is not None:
                    if not ("v", b, prev[0]) in emitted:
                        force(lambda k, kk=prev[0]: k == ("v", b, kk))
                    emit_av(*prev)

                def finish():
                    for h in sorted(bank_ots):
                        ot, jb, nb = bank_ots[h]
                        for jj in range(nb):
                            pt = pop.tile([128, 65], F32, tag="po0",
                                          name="pt")
                            nc.tensor.transpose(
                                pt[:], ot[0:65, jj * 128:(jj + 1) * 128],
                                identf_t[0:65, 0:65])
                            normalize_tile(b, h, jb + jj, pt[:])
                return finish

            # --- orchestration ------------------------------------------
            # Spans alternate heads: h1 re-uses h0's projections, so the
            # ACT work per projection deadline doubles and background
            # projection quanta fit inside the span slack.
            # prologue: QK chunk 0 + V tiles 0-1 of b0 (drains split
            # ACT/DVE to shorten the startup chain)
            pj0 = stripp.tile([128, 1024], F32, tag="strip",
                              name="pj0")
            # all Q matmuls before all K: the wk DMA lands after wq, and
            # interleaving would head-of-line block Q matmuls behind K(d0)
            xt0, tw0, base0 = x_slice(0, 0, 256)
            for d in range(ND):
                nc.tensor.matmul(
                    pj0[:, 0:256], wqk_t[:, d * 128:(d + 1) * 128],
                    xt0[:, d * tw0 + base0:d * tw0 + base0 + 256],
                    start=(d == 0), stop=(d == ND - 1))
            for d in range(ND):
                nc.tensor.matmul(
                    pj0[:, 256:512],
                    wqk_t[:, ND * 128 + d * 128:ND * 128 + (d + 1) * 128],
                    xt0[:, d * tw0 + base0:d * tw0 + base0 + 256],
                    start=False, stop=(d == ND - 1))
            qk_chunk_drain(0, 0, 0, pj0, on_act=True, hs=(0,))
            qk_chunk_drain(0, 0, 1, pj0, on_act=True, hs=(0,))
            qk_chunk_drain(0, 0, 0, pj0, on_act=False, hs=(1,))
            qk_chunk_drain(0, 0, 1, pj0, on_act=False, hs=(1,))
            for ki in (0, 1):
                queue_v(0, ki)

            # chunk 1 matmuls inline during the ACT-idle startup;
            # its drains go to the background queue
            pj1 = stripp.tile([128, 1024], F32, tag="strip",
                              name="pj1")
            qk_chunk_mm(0, 1, 0, ND, pj1)
            for part in range(2):
                for h in range(2):
                    def c1drain(on_act=False, p=part, hh=h):
                        qk_chunk_drain(0, 1, p, pj1, on_act=on_act,
                                       hs=(hh,))
                    drain_fns[(0, 1, part, h)] = c1drain
                    bg_add(("qkd", 0, 1, part, h), 0, 400, c1drain)
            for q in range(4):
                emitted.add(("qkm", 0, 1, q))


            for g in (1, 2, 3):
                queue_dma(("xg", g), lambda g=g: load_group(g))
            for c in (2, 3):
                queue_qk(0, c)
            for ki in (2, 3, 4, 5):
                queue_v(0, ki)
            for c in (4, 5):
                queue_qk(0, c)
            for ki in (6, 7, 8, 9):
                queue_v(0, ki)
            for c in (6, 7):
                queue_qk(0, c)
            for ki in range(10, NJ):
                queue_v(0, ki)

            def need_qk(b, cs):
                force(lambda k: k[0] == "qkm" and k[1] == b
                      and k[2] in cs)
                # bulk-forced drains: ACT is idle while it waits for
                # these, so alternate them between ACT and DVE
                n = 0
                for c in cs:
                    for part in range(2):
                        for h in range(2):
                            key = ("qkd", b, c, part, h)
                            if key in emitted or key not in by_key:
                                continue
                            emitted.add(key)
                            drain_fns[(b, c, part, h)](on_act=n % 2 == 0)
                            n += 1

            # batch 0 (merged-head spans, qw <= 512)
            cr = attention_span(0, 0, 256)
            need_qk(0, (1,))
            cr = attention_span(0, 256, 256, carry=cr)
            need_qk(0, (2, 3))
            cr = attention_span(0, 512, 512, carry=cr)
            # queue b1 inputs + b1 projection work mid-flight
            queue_dma(("x1a", 0), load_b1_first)
            queue_dma(("aug1", 0), load_aug_b1)
            for g in (5, 6, 7):
                queue_dma(("xg", g), lambda g=g: load_group(g))
            for c in (0, 1, 2, 3):
                queue_qk(1, c)
            for ki in (0, 1, 2, 3):
                queue_v(1, ki)
            for c in (4, 5, 6, 7):
                queue_qk(1, c)
            for ki in range(4, NJ):
                queue_v(1, ki)
            need_qk(0, (4, 5))
            cr = attention_span(0, 1024, 512, carry=cr)
            need_qk(0, (6, 7))
            cr = attention_span(0, 1536, 512, carry=cr)

            # batch 1
            force(lambda k: k[0] in ("x1a", "aug1"))
            need_qk(1, (0, 1))
            cr = attention_span(1, 0, 512, carry=cr)
            need_qk(1, (2, 3))
            cr = attention_span(1, 512, 512, carry=cr)
            need_qk(1, (4, 5))
            cr = attention_span(1, 1024, 512, carry=cr)
            need_qk(1, (6, 7))
            cr = attention_span(1, 1536, 512, last_span=True, carry=cr)
            cr()
            force(lambda k: True)

    nc.compile()
    return nc


def _get_program():
    if "nc" not in _CACHE:
        _CACHE["nc"] = _build_program()
    return _CACHE["nc"]


def _prep_inputs(input, attention_mask, W_Q, W_K, W_V):
    f16 = np.float16

    x = np.asarray(input, dtype=np.float32).reshape(T, D)
    xT = np.ascontiguousarray(x.T).astype(f16)             # [D, T]
    # [d, p, g, tloc]
    xr = xT.reshape(ND, 128, B * 4, 512).transpose(1, 2, 0, 3)
    # xr: [p, g, d, tloc]
    xh = {}
    for b in range(B):
        g0 = xr[:, b * 4]                                    # [p, d, 512]
        xh[f"xh0a{b}"] = np.ascontiguousarray(
            g0[:, :, 0:256].reshape(128, ND * 256))
        xh[f"xh0b{b}"] = np.ascontiguousarray(
            g0[:, :, 256:512].reshape(128, ND * 256))
    rest = np.concatenate([xr[:, g] for g in (1, 2, 3, 5, 6, 7)],
                          axis=1)                            # [p, 6*d, 512]
    xhr = np.ascontiguousarray(rest.reshape(128, 6 * ND * 512))

    import ml_dtypes
    mask = np.asarray(attention_mask).astype(np.float32).reshape(1, T)
    kaug = ((mask - 1.0) * NEG_BIG).astype(f16)
    qi = np.arange(128)
    trilb = np.where(qi[None, :] >= qi[:, None], 1.0,
                     0.0).astype(ml_dtypes.bfloat16)   # keep[k,q]: q >= k
    identf = np.eye(128, dtype=np.float32)

    common = {
        **xh, "xhr": xhr, "kaug": kaug, "trilb": trilb,
        "identf": identf,
    }

    def pack_w(Wm, sl):
        wc = np.asarray(Wm, np.float32)[sl, :].astype(f16)  # [128e, D]
        return wc.reshape(128, ND, 128).transpose(2, 1, 0)   # [p, d, e]

    in_maps = []
    for c in range(N_CORES):
        sl = slice(c * E, (c + 1) * E)
        wq = pack_w(W_Q, sl)
        wk = pack_w(W_K, sl)
        wv = pack_w(W_V, sl)
        in_maps.append({
            **common,
            "wqk": np.ascontiguousarray(
                np.concatenate([wq, wk], axis=1).reshape(128, 2 * ND * 128)),
            "wv": np.ascontiguousarray(wv.reshape(128, ND * 128)),
        })
    return in_maps


def kernel(input, attention_mask, W_Q, W_K, W_V):
    from concourse.bass_utils import run_bass_kernel_spmd

    nc = _get_program()
    in_maps = _prep_inputs(input, attention_mask, W_Q, W_K, W_V)
    res = run_bass_kernel_spmd(nc, in_maps, list(range(N_CORES)))
    return np.concatenate([res.results[c]["out"] for c in range(N_CORES)],
                          axis=2)
